# revision 5
# baseline (speedup 1.0000x reference)
"""Trainium2 Bass kernel for nn_DynamicShortConvolution.

Reference computation (per token t, channel d):
    h    = silu(x @ w1)                       # [T, H]
    flat = h @ w2 + b2                        # [T, D*W]
    k    = flat.reshape(T, D, W)
    out[t, d] = silu(sum_w k[t, d, w] * x[t - (W-1) + w, d])

Sharding: 8 cores, each one (batch, half-of-T) shard of 2048 tokens plus a
3-token left halo.  Per-core tensors are TRANSPOSED ([D, T], channels on SBUF
partitions) so the causal shift is a free-dim offset and both matmuls run
without on-device transposes.

Schedule (v3):
  - DMA order w1, b2, x(16 tiles), w2(8 chunks) so mm1 starts ~4us in.
  - mm1 is dt-OUTER: each arriving x tile feeds one 8-matmul burst
    accumulating into 2 resident [128,2048] PSUM tiles; mm1 finishes right
    after the last x tile lands.
  - w2 is stored dt-major so mm2 group (dt,pi) only needs its own w2 chunk.
  - mm2 elementwise: NO GPSIMD (its SBUF-port sharing slows concurrent
    2-port DVE ops 2-4x, measured).  Per group (1024 tokens):
      ACT : two paired FD2048 PSUM->SBUF copies (tap evac, no bias),
            one FD2048 silu per group PAIR
      DVE : 4x stt (kb+bias)*x products (bf16 2x mode), 2 adds
      DMA : odd-shift x copy (keeps products 4B-aligned), output per dt
  - 1-group software pipeline skew so no engine stalls on same-group
    producers.
"""

import numpy as np

# Problem constants (hardcoded per harness contract).
B, T, D, H, W = 4, 4096, 2048, 256, 4
HALO = W - 1
N_CORES = 8
TOK = (B * T) // N_CORES  # tokens per core = 2048


def _build_nc(tok, d, h, xstride):
    import concourse.bass as bass
    import concourse.bacc as bacc
    import concourse.mybir as mybir
    import concourse.tile as tile

    f32 = mybir.dt.float32
    bf16 = mybir.dt.bfloat16
    AF = mybir.ActivationFunctionType
    ALU = mybir.AluOpType

    n_dt = d // 128        # 16 d tiles
    n_hc = h // 128        # 2 h tiles
    P = 1024               # tokens per mm2 group
    n_pi = tok // P        # 2
    NG = n_dt * n_pi       # 32 groups

    nc = bacc.Bacc()

    # DRAM I/O (host-prepared layouts)
    xT = nc.declare_dram_parameter("xT", [n_dt, 128, xstride], bf16, isOutput=False)
    # w1d[p, dt*h + j] = w1[dt*128+p, j]
    w1d = nc.declare_dram_parameter("w1d", [128, n_dt * h], bf16, isOutput=False)
    # w2d[p, dt*1024 + hc*512 + w*128 + c] = w2[hc*128+p, (dt*128+c)*W + w]
    w2d = nc.declare_dram_parameter("w2d", [128, n_dt * 1024], bf16, isOutput=False)
    # b2d[p, dt*W + w] = b2[(dt*128+p)*W + w]
    b2d = nc.declare_dram_parameter("b2d", [128, n_dt * W], f32, isOutput=False)
    outT = nc.declare_dram_parameter("outT", [n_dt, 128, tok], bf16, isOutput=True)

    with tile.TileContext(nc) as tc:
        with (
            tc.tile_pool(name="resident", bufs=1) as rpool,
            tc.tile_pool(name="work", bufs=2) as wpool,
            tc.tile_pool(name="psum", bufs=2, space="PSUM") as ppool,
        ):
            # ---- resident tiles ----
            xT_sb = rpool.tile([128, n_dt * xstride], bf16, tag="xT")
            w1_sb = rpool.tile([128, n_dt * h], bf16, tag="w1")
            w2_sb = rpool.tile([128, n_dt * 1024], bf16, tag="w2")
            b2_sb = rpool.tile([128, n_dt * W], f32, tag="b2")
            hT_sb = rpool.tile([128, n_hc * tok], bf16, tag="hT")

            # ---- DMA issue order: w1, b2, x tiles, w2 chunks ----
            nc.sync.dma_start(w1_sb[:, :], w1d[:, :])
            nc.sync.dma_start(b2_sb[:, :], b2d[:, :])
            for dt in range(n_dt):
                nc.sync.dma_start(
                    xT_sb[:, dt * xstride:(dt + 1) * xstride], xT[dt])
            for c in range(8):  # 2 dt per chunk
                nc.sync.dma_start(
                    w2_sb[:, c * 2048:(c + 1) * 2048],
                    w2d[:, c * 2048:(c + 1) * 2048])

            def x_slice(dt, col, n):
                return xT_sb[:, dt * xstride + col: dt * xstride + col + n]

            # ---- mm1 (dt-outer): hT = silu(w1.T @ xT) ----
            # psA = hc0 (tokens 0..2047), psB = hc1
            psA1 = ppool.tile([128, 2 * P], f32, tag="ps", name="psA1")
            psB1 = ppool.tile([128, 2 * P], f32, tag="ps", name="psB1")
            for dt in range(n_dt):
                for hc in range(n_hc):
                    pt = psA1 if hc == 0 else psB1
                    for tci in range(4):
                        nc.tensor.matmul(
                            pt[:, tci * 512:(tci + 1) * 512],
                            w1_sb[:, dt * h + hc * 128: dt * h + hc * 128 + 128],
                            x_slice(dt, HALO + tci * 512, 512),
                            start=(dt == 0), stop=(dt == n_dt - 1),
                        )
            nc.scalar.activation(hT_sb[:, 0:tok], psA1[:], AF.Silu)
            nc.scalar.activation(hT_sb[:, tok:2 * tok], psB1[:], AF.Silu)

            # ---- mm2 + conv + silu, software pipeline over 32 groups ----
            st = [None] * NG

            def bias(dt, w):
                return b2_sb[:, dt * W + w: dt * W + w + 1]

            for g in range(NG + 2):
                # ---- stage A (group g): matmuls + evac pair copies ----
                if g < NG:
                    dt, pi = divmod(g, n_pi)
                    j0 = pi * P
                    # psA = [k0 | k1], psB = [k2 | k3]
                    psA = ppool.tile([128, 2 * P], f32, tag="ps",
                                     name=f"psA_{g}")
                    psB = ppool.tile([128, 2 * P], f32, tag="ps",
                                     name=f"psB_{g}")
                    for w in range(W):
                        pt = psA if w < 2 else psB
                        c0 = (w % 2) * P
                        for hc in range(n_hc):
                            for tcj in range(2):
                                nc.tensor.matmul(
                                    pt[:, c0 + tcj * 512: c0 + (tcj + 1) * 512],
                                    w2_sb[:, dt * 1024 + hc * 512 + w * 128:
                                          dt * 1024 + hc * 512 + w * 128 + 128],
                                    hT_sb[:, hc * tok + j0 + tcj * 512:
                                          hc * tok + j0 + (tcj + 1) * 512],
                                    start=(hc == 0), stop=(hc == n_hc - 1),
                                )
                    # ACT: paired evac copies (no bias; bias rides in stt)
                    kb = wpool.tile([128, 4 * P], bf16, tag="kb",
                                    name=f"kb_{g}")
                    nc.scalar.activation(kb[:, 0:2 * P], psA[:], AF.Copy)
                    nc.scalar.activation(kb[:, 2 * P:4 * P], psB[:], AF.Copy)
                    # DMA: odd-shift x window for taps 1,3 (4B alignment)
                    xs = wpool.tile([128, P + 4], bf16, tag="xs",
                                    name=f"xs_{g}")
                    nc.sync.dma_start(xs[:, 0:P + 2], x_slice(dt, j0 + 1, P + 2))
                    st[g] = dict(dt=dt, j0=j0, kb=kb, xs=xs)

                # ---- stage B (group g-1): products + adds (DVE) ----
                if 0 <= g - 1 < NG:
                    gp = g - 1
                    s = st[gp]
                    dt1, j1 = s["dt"], s["j0"]
                    kb, xs = s["kb"], s["xs"]
                    m = wpool.tile([128, 4 * P], bf16, tag="m", name=f"m_{gp}")
                    nc.vector.scalar_tensor_tensor(
                        m[:, 0:P], kb[:, 0:P], bias(dt1, 0),
                        x_slice(dt1, j1 + 0, P), op0=ALU.add, op1=ALU.mult)
                    nc.vector.scalar_tensor_tensor(
                        m[:, P:2 * P], kb[:, P:2 * P], bias(dt1, 1),
                        xs[:, 0:P], op0=ALU.add, op1=ALU.mult)
                    nc.vector.scalar_tensor_tensor(
                        m[:, 2 * P:3 * P], kb[:, 2 * P:3 * P], bias(dt1, 2),
                        x_slice(dt1, j1 + 2, P), op0=ALU.add, op1=ALU.mult)
                    nc.vector.scalar_tensor_tensor(
                        m[:, 3 * P:4 * P], kb[:, 3 * P:4 * P], bias(dt1, 3),
                        xs[:, 2:P + 2], op0=ALU.add, op1=ALU.mult)
                    aa = wpool.tile([128, 2 * P], bf16, tag="aa", name=f"aa_{gp}")
                    nc.vector.tensor_add(aa[:], m[:, 0:2 * P], m[:, 2 * P:4 * P])
                    # acc halves: pair groups (dt,0)+(dt,1) share one tile
                    if pi_of := gp % 2:
                        accp = s["accp"] = st[gp - 1]["accp"]
                    else:
                        accp = s["accp"] = wpool.tile(
                            [128, 2 * P], bf16, tag="accp", name=f"accp_{gp}")
                    nc.vector.tensor_add(accp[:, pi_of * P:(pi_of + 1) * P],
                                         aa[:, 0:P], aa[:, P:2 * P])
                    # ---- stage C (per pair): silu + DMA out full dt row ----
                    if pi_of == 1:
                        dt3 = dt1
                        ot = wpool.tile([128, 2 * P], bf16, tag="ot",
                                        name=f"ot_{gp}")
                        nc.scalar.activation(ot[:], accp[:], AF.Silu)
                        nc.sync.dma_start(outT[dt3, :, :], ot[:])
                        st[gp] = st[gp - 1] = None
    nc.compile()
    return nc


def _prep_shards(x, w1, w2, b2, tok, d, h, halo, xstride):
    """Host-side shard prep. Returns list of per-core in_maps."""
    import ml_dtypes
    bf16 = ml_dtypes.bfloat16

    n_dt = d // 128
    b, t, _ = x.shape
    shards_per_batch = (b * t // tok) // b

    # w1d[p, dt*h + j] = w1[dt*128+p, j]
    w1_r = np.ascontiguousarray(
        w1.reshape(n_dt, 128, h).transpose(1, 0, 2).reshape(128, n_dt * h)
    ).astype(bf16)
    # w2d[p, dt*1024 + hc*512 + w*128 + c] = w2[hc*128+p, (dt*128+c)*W + w]
    w2_4d = w2.reshape(2, 128, d, W)              # [hc, p, dcol, w]
    w2_5d = w2_4d.reshape(2, 128, n_dt, 128, W)   # [hc, p, dt, c, w]
    w2_r = np.ascontiguousarray(
        w2_5d.transpose(1, 2, 0, 4, 3)            # [p, dt, hc, w, c]
        .reshape(128, n_dt * 1024)).astype(bf16)
    # b2d[p, dt*W + w] = b2[(dt*128+p)*W + w]
    b2_r = np.ascontiguousarray(
        b2.reshape(n_dt, 128, W).transpose(1, 0, 2).reshape(128, n_dt * W)
    ).astype(np.float32)

    in_maps = []
    for core in range(N_CORES):
        bi, half = divmod(core, shards_per_batch)
        t0 = half * tok
        xh = np.zeros((tok + halo, d), np.float32)
        lo = max(t0 - halo, 0)
        xh[halo - (t0 - lo):] = x[bi, lo: t0 + tok]
        xTc = np.zeros((n_dt, 128, xstride), bf16)
        xTc[:, :, : tok + halo] = (
            xh.T.astype(bf16).reshape(n_dt, 128, tok + halo))
        in_maps.append({
            "xT": xTc, "w1d": w1_r, "w2d": w2_r, "b2d": b2_r})
    return in_maps


_NC_CACHE = {}


def kernel(x, w1, w2, b2, trace=False):
    from concourse.bass_utils import run_bass_kernel_spmd

    tok, d, h = TOK, D, H
    xstride = tok + HALO + 1  # even -> keeps bf16 4B alignment per dtile
    key = (tok, d, h)
    if key not in _NC_CACHE:
        _NC_CACHE[key] = _build_nc(tok, d, h, xstride)
    nc = _NC_CACHE[key]

    in_maps = _prep_shards(
        np.asarray(x, np.float32), np.asarray(w1, np.float32),
        np.asarray(w2, np.float32), np.asarray(b2, np.float32),
        tok, d, h, HALO, xstride)

    res = run_bass_kernel_spmd(nc, in_maps, core_ids=list(range(N_CORES)),
                               trace=trace)
    kernel.last_result = res

    shards_per_batch = (B * T // tok) // B
    out = np.empty((B, T, D), np.float32)
    for core in range(N_CORES):
        bi, half = divmod(core, shards_per_batch)
        oT = res.results[core]["outT"]  # [n_dt, 128, tok]
        out[bi, half * tok:(half + 1) * tok] = (
            oT.reshape(d, tok).T.astype(np.float32))
    return out


# revision 6
# speedup vs baseline: 1.1732x; 1.1732x over previous
"""Trainium2 Bass kernel for nn_DynamicShortConvolution.

Reference computation (per token t, channel d):
    h    = silu(x @ w1)                       # [T, H]
    flat = h @ w2 + b2                        # [T, D*W]
    k    = flat.reshape(T, D, W)
    out[t, d] = silu(sum_w k[t, d, w] * x[t - (W-1) + w, d])

Sharding: 8 cores, each one (batch, half-of-T) shard of 2048 tokens plus a
3-token left halo.  Per-core tensors are TRANSPOSED ([D, T], channels on SBUF
partitions) so the causal shift is a free-dim offset and both matmuls run
without on-device transposes.

Schedule (v4) based on measured per-op costs:
  - DMA order w1, b2, x(16 tiles), w2(8 chunks); mm1 is dt-OUTER so it
    overlaps the x load and finishes right after the last x tile lands.
  - w2 stored dt-major so mm2 group (dt,pi) needs only its own chunk.
  - mm2 elementwise, per 1024-token group, engine-balanced:
      DVE : stt taps 1,3 straight from PSUM (evac+bias+product in one op,
            1x but errata/alignment/contention-immune), TT products for
            taps 0,2 (bf16 2x), one pairwise add
      ACT : bias-evac taps 0,2 (FD1024; FD2048 measured slower), silu
      PE  : 3-term identity-matmul reduce (m0 + m2 + a13) accumulated in
            PSUM -- the tensor engine replaces the DVE/GPSIMD add tree
      GPS : nothing (its SBUF-port sharing slows concurrent DVE 2x ops)
  - acc reuses the k2 PSUM region (subtile deps) so everything fits in 8
    banks; 2-iteration software-pipeline skew keeps all queues stall-free.
"""

import numpy as np

# Problem constants (hardcoded per harness contract).
B, T, D, H, W = 4, 4096, 2048, 256, 4
HALO = W - 1
N_CORES = 8
TOK = (B * T) // N_CORES  # tokens per core = 2048


def _build_nc(tok, d, h, xstride):
    import concourse.bass as bass
    import concourse.bacc as bacc
    import concourse.mybir as mybir
    import concourse.tile as tile

    f32 = mybir.dt.float32
    bf16 = mybir.dt.bfloat16
    AF = mybir.ActivationFunctionType
    ALU = mybir.AluOpType

    n_dt = d // 128        # 16 d tiles
    n_hc = h // 128        # 2 h tiles
    P = 1024               # tokens per mm2 group
    n_pi = tok // P        # 2
    NG = n_dt * n_pi       # 32 groups

    nc = bacc.Bacc()

    # DRAM I/O (host-prepared layouts)
    xT = nc.declare_dram_parameter("xT", [n_dt, 128, xstride], bf16, isOutput=False)
    # w1d[p, dt*h + j] = w1[dt*128+p, j]
    w1d = nc.declare_dram_parameter("w1d", [128, n_dt * h], bf16, isOutput=False)
    # w2d[p, dt*1024 + hc*512 + w*128 + c] = w2[hc*128+p, (dt*128+c)*W + w]
    w2d = nc.declare_dram_parameter("w2d", [128, n_dt * 1024], bf16, isOutput=False)
    # b2d[p, dt*W + w] = b2[(dt*128+p)*W + w]
    b2d = nc.declare_dram_parameter("b2d", [128, n_dt * W], f32, isOutput=False)
    # identity for PE reduce matmuls
    idd = nc.declare_dram_parameter("idd", [128, 128], bf16, isOutput=False)
    outT = nc.declare_dram_parameter("outT", [n_dt, 128, tok], bf16, isOutput=True)

    with tile.TileContext(nc) as tc:
        with (
            tc.tile_pool(name="resident", bufs=1) as rpool,
            tc.tile_pool(name="work", bufs=3) as wpool,
            tc.tile_pool(name="psum", bufs=1, space="PSUM") as ppool,
        ):
            # ---- resident tiles ----
            xT_sb = rpool.tile([128, n_dt * xstride], bf16, tag="xT")
            w1_sb = rpool.tile([128, n_dt * h], bf16, tag="w1")
            w2_sb = rpool.tile([128, n_dt * 1024], bf16, tag="w2")
            b2_sb = rpool.tile([128, n_dt * W], f32, tag="b2")
            id_sb = rpool.tile([128, 128], bf16, tag="idd")
            hT_sb = rpool.tile([128, n_hc * tok], bf16, tag="hT")

            # ---- DMA issue order: w1, b2, id, x tiles, w2 chunks ----
            nc.sync.dma_start(w1_sb[:, :], w1d[:, :])
            nc.sync.dma_start(b2_sb[:, :], b2d[:, :])
            nc.sync.dma_start(id_sb[:, :], idd[:, :])
            for dt in range(n_dt):
                nc.sync.dma_start(
                    xT_sb[:, dt * xstride:(dt + 1) * xstride], xT[dt])
            for c in range(8):  # 2 dt per chunk
                nc.sync.dma_start(
                    w2_sb[:, c * 2048:(c + 1) * 2048],
                    w2d[:, c * 2048:(c + 1) * 2048])

            def x_slice(dt, col, n):
                return xT_sb[:, dt * xstride + col: dt * xstride + col + n]

            # Two resident PSUM pair-tiles (4 banks each = all 8 banks).
            # ps13 holds [k1|k3]; ps02 holds [k0|k2]; the group reduce
            # (acc) time-shares ps02's k2 half.
            ps13 = ppool.tile([128, 2 * P], f32, tag="ps13")
            ps02 = ppool.tile([128, 2 * P], f32, tag="ps02")

            # ---- mm1 (dt-outer): hT = silu(w1.T @ xT) ----
            for dt in range(n_dt):
                for hc in range(n_hc):
                    pt = ps13 if hc == 0 else ps02
                    for tci in range(4):
                        nc.tensor.matmul(
                            pt[:, tci * 512:(tci + 1) * 512],
                            w1_sb[:, dt * h + hc * 128: dt * h + hc * 128 + 128],
                            x_slice(dt, HALO + tci * 512, 512),
                            start=(dt == 0), stop=(dt == n_dt - 1),
                        )
            for half in range(2):
                nc.scalar.activation(
                    hT_sb[:, half * P:(half + 1) * P],
                    ps13[:, half * P:(half + 1) * P], AF.Silu)
                nc.scalar.activation(
                    hT_sb[:, tok + half * P: tok + (half + 1) * P],
                    ps02[:, half * P:(half + 1) * P], AF.Silu)

            # ---- mm2 + conv + silu, software pipeline over 32 groups ----
            st = [None] * NG

            def bias(dt, w):
                return b2_sb[:, dt * W + w: dt * W + w + 1]

            def tap_mms(g, w, pt, c0):
                dt, pi = divmod(g, n_pi)
                j0 = pi * P
                for hc in range(n_hc):
                    for tcj in range(2):
                        nc.tensor.matmul(
                            pt[:, c0 + tcj * 512: c0 + (tcj + 1) * 512],
                            w2_sb[:, dt * 1024 + hc * 512 + w * 128:
                                  dt * 1024 + hc * 512 + w * 128 + 128],
                            hT_sb[:, hc * tok + j0 + tcj * 512:
                                  hc * tok + j0 + (tcj + 1) * 512],
                            start=(hc == 0), stop=(hc == n_hc - 1),
                        )

            for g in range(NG + 2):
                # ---- PE: 3-term identity reduce for group g-2 into the
                # (just-freed) k2 half of ps02: acc = m0 + m2 + a13
                if 0 <= g - 2 < NG:
                    s2 = st[g - 2]
                    for ci, term in enumerate((s2["m0"], s2["m2"], s2["a13"])):
                        for c in range(2):
                            nc.tensor.matmul(
                                ps02[:, P + c * 512: P + (c + 1) * 512],
                                id_sb[:, :],
                                term[:, c * 512:(c + 1) * 512],
                                start=(ci == 0), stop=(ci == 2),
                            )

                # ---- PE: tap matmuls for group g (w1, w3 first) ----
                if g < NG:
                    dt, pi = divmod(g, n_pi)
                    j0 = pi * P
                    tap_mms(g, 1, ps13, 0)
                    tap_mms(g, 3, ps13, P)
                    tap_mms(g, 0, ps02, 0)
                    tap_mms(g, 2, ps02, P)

                # ---- ACT: silu of group g-2's acc, then bias-evacs of g ----
                if 0 <= g - 2 < NG:
                    dt2, pi2 = divmod(g - 2, n_pi)
                    ot = wpool.tile([128, P], bf16, tag="ot", name=f"ot_{g-2}")
                    nc.scalar.activation(ot[:], ps02[:, P:2 * P], AF.Silu)
                    nc.sync.dma_start(
                        outT[dt2, :, pi2 * P:(pi2 + 1) * P], ot[:])
                if g < NG:
                    kb = wpool.tile([128, 2 * P], bf16, tag="kb", name=f"kb_{g}")
                    nc.scalar.add(kb[:, 0:P], ps02[:, 0:P], bias(dt, 0))
                    nc.scalar.add(kb[:, P:2 * P], ps02[:, P:2 * P], bias(dt, 2))
                    st[g] = dict(dt=dt, j0=j0, kb=kb)

                # ---- DVE: a13(g-1), products m0/m2 (g-1), stt taps 1,3 (g) --
                if 0 <= g - 1 < NG:
                    s1 = st[g - 1]
                    dt1, j1, kb1 = s1["dt"], s1["j0"], s1["kb"]
                    a13 = wpool.tile([128, P], bf16, tag="a13", name=f"a13_{g-1}")
                    nc.vector.tensor_add(a13[:], s1["m1"][:, 0:P],
                                         s1["m1"][:, P:2 * P])
                    s1["a13"] = a13
                    m0 = wpool.tile([128, P], bf16, tag="m0", name=f"m0_{g-1}")
                    nc.vector.tensor_mul(m0[:], kb1[:, 0:P],
                                         x_slice(dt1, j1 + 0, P))
                    m2 = wpool.tile([128, P], bf16, tag="m2", name=f"m2_{g-1}")
                    nc.vector.tensor_mul(m2[:], kb1[:, P:2 * P],
                                         x_slice(dt1, j1 + 2, P))
                    s1["m0"], s1["m2"] = m0, m2
                if g < NG:
                    m1 = wpool.tile([128, 2 * P], bf16, tag="m1", name=f"m1_{g}")
                    nc.vector.scalar_tensor_tensor(
                        m1[:, 0:P], ps13[:, 0:P], bias(dt, 1),
                        x_slice(dt, j0 + 1, P), op0=ALU.add, op1=ALU.mult)
                    nc.vector.scalar_tensor_tensor(
                        m1[:, P:2 * P], ps13[:, P:2 * P], bias(dt, 3),
                        x_slice(dt, j0 + 3, P), op0=ALU.add, op1=ALU.mult)
                    st[g]["m1"] = m1
    nc.compile()
    return nc


def _prep_shards(x, w1, w2, b2, tok, d, h, halo, xstride):
    """Host-side shard prep. Returns list of per-core in_maps."""
    import ml_dtypes
    bf16 = ml_dtypes.bfloat16

    n_dt = d // 128
    b, t, _ = x.shape
    shards_per_batch = (b * t // tok) // b

    # w1d[p, dt*h + j] = w1[dt*128+p, j]
    w1_r = np.ascontiguousarray(
        w1.reshape(n_dt, 128, h).transpose(1, 0, 2).reshape(128, n_dt * h)
    ).astype(bf16)
    # w2d[p, dt*1024 + hc*512 + w*128 + c] = w2[hc*128+p, (dt*128+c)*W + w]
    w2_4d = w2.reshape(2, 128, d, W)              # [hc, p, dcol, w]
    w2_5d = w2_4d.reshape(2, 128, n_dt, 128, W)   # [hc, p, dt, c, w]
    w2_r = np.ascontiguousarray(
        w2_5d.transpose(1, 2, 0, 4, 3)            # [p, dt, hc, w, c]
        .reshape(128, n_dt * 1024)).astype(bf16)
    # b2d[p, dt*W + w] = b2[(dt*128+p)*W + w]
    b2_r = np.ascontiguousarray(
        b2.reshape(n_dt, 128, W).transpose(1, 0, 2).reshape(128, n_dt * W)
    ).astype(np.float32)
    id_r = np.eye(128, dtype=np.float32).astype(bf16)

    in_maps = []
    for core in range(N_CORES):
        bi, half = divmod(core, shards_per_batch)
        t0 = half * tok
        xh = np.zeros((tok + halo, d), np.float32)
        lo = max(t0 - halo, 0)
        xh[halo - (t0 - lo):] = x[bi, lo: t0 + tok]
        xTc = np.zeros((n_dt, 128, xstride), bf16)
        xTc[:, :, : tok + halo] = (
            xh.T.astype(bf16).reshape(n_dt, 128, tok + halo))
        in_maps.append({
            "xT": xTc, "w1d": w1_r, "w2d": w2_r, "b2d": b2_r, "idd": id_r})
    return in_maps


_NC_CACHE = {}


def kernel(x, w1, w2, b2, trace=False):
    from concourse.bass_utils import run_bass_kernel_spmd

    tok, d, h = TOK, D, H
    xstride = tok + HALO + 1  # even -> keeps bf16 4B alignment per dtile
    key = (tok, d, h)
    if key not in _NC_CACHE:
        _NC_CACHE[key] = _build_nc(tok, d, h, xstride)
    nc = _NC_CACHE[key]

    in_maps = _prep_shards(
        np.asarray(x, np.float32), np.asarray(w1, np.float32),
        np.asarray(w2, np.float32), np.asarray(b2, np.float32),
        tok, d, h, HALO, xstride)

    res = run_bass_kernel_spmd(nc, in_maps, core_ids=list(range(N_CORES)),
                               trace=trace)
    kernel.last_result = res

    shards_per_batch = (B * T // tok) // B
    out = np.empty((B, T, D), np.float32)
    for core in range(N_CORES):
        bi, half = divmod(core, shards_per_batch)
        oT = res.results[core]["outT"]  # [n_dt, 128, tok]
        out[bi, half * tok:(half + 1) * tok] = (
            oT.reshape(d, tok).T.astype(np.float32))
    return out
